# revision 22
# baseline (speedup 1.0000x reference)
"""Trainium2 Bass kernel for nn_CausalRSSM: data-parallel over batch on 8 cores.

Layout: all activations transposed (feature dim on SBUF partitions, batch on
free dim), so every layer is matmul(lhsT=W[K,M], rhs=X^T[K,B]) with weights in
their natural [in, out] storage order. Host pre-transposes features/eps,
absorbs the adjacency A into mech_w1, packs the 496 MI pair-MLPs into
2-pairs-per-matmul blocks (sparse lhsT rows vs the shared z rhsm, block-diag
second layer), and finishes scalar loss reductions from per-core partial sums.
"""
import sys
import numpy as np

for _p in ("/opt/trn_rl_repo",):
    if _p not in sys.path:
        sys.path.append(_p)

import concourse.bass as bass
import concourse.mybir as mybir
import concourse.tile as tile
from concourse import bacc
from concourse.bass_utils import run_bass_kernel_spmd

F32 = mybir.dt.float32
F32R = mybir.dt.float32r
AF = mybir.ActivationFunctionType
ALU = mybir.AluOpType
PS = bass.MemorySpace.PSUM

# problem dims (hardcoded per spec)
B, F, S, H = 4096, 512, 32, 512
H2, CH = 256, 64
P = S * (S - 1) // 2          # 496 pairs
NCORES = 8
BL = B // NCORES              # 512 rows per core
NBLK = P // 2                 # 248 two-pair blocks
GRP = 8                       # MI blocks per streamed weight group
NGRP = NBLK // GRP            # 31
LEAK = 0.2
TEMP = 0.1

_CACHE = {}


def _build_nc():
    nc = bacc.Bacc(None)
    d = {}
    def din(name, shape, dt=F32R):
        d[name] = nc.declare_dram_parameter(name, list(shape), dt, isOutput=False)
    def dout(name, shape, dt=F32R):
        d[name] = nc.declare_dram_parameter(name, list(shape), dt, isOutput=True)

    din("xT", (F, BL))              # features^T
    din("epsT", (S, BL))
    din("ew1", (F, H))              # enc_w1 [K,M]
    din("ew2", (F, 2 * S))
    din("eb1", (128, 4), F32)       # enc_b1 packed per M-tile column
    din("eb2", (S, 3), F32)         # cols: mean bias, ls bias, 2*ls bias
    din("m1w", (S + 1, S * H2))     # mech layer1 lhsT (+bias row), col blk (i,c)
    din("m2w", (128, 2 * S * S))    # mech layer2 lhsT, col blk (i,c) x 32 cols
    din("mb2", (S, 1), F32)         # mech_b2
    din("dw1", (S, H))
    din("dw2", (H, F))
    din("db1", (128, 4), F32)
    din("db2", (128, 4), F32)
    din("mi1w", (NGRP, S + 1, GRP * 128))   # MI layer1 lhsT groups (+bias row)
    din("mi2w", (NGRP, 128, GRP * 128))     # MI layer2 block-diag lhsT groups
    din("b2p", (128, NBLK), F32)            # mi_b2 packed 2 pairs per col

    dout("meanT", (S, BL))
    dout("stdT", (S, BL))
    dout("zcT", (S, BL))
    dout("reconT", (F, BL))
    dout("sT", (128, NBLK), F32)    # sum_b lrelu(m2) per (pair-of-2, ch)
    dout("klp", (S, 3), F32)        # sum mean^2, sum std^2, sum log_std
    dout("rsq", (128, 4), F32)      # sum (recon - x)^2 per dec2 M-tile

    with tile.TileContext(nc) as tc:
        with (
            tc.tile_pool(name="cst", bufs=1) as cp,
            tc.tile_pool(name="g1", bufs=3) as g1p,
            tc.tile_pool(name="g2", bufs=3) as g2p,
            tc.tile_pool(name="m1t", bufs=4) as m1p,
            tc.tile_pool(name="scr", bufs=3) as scp,
            tc.tile_pool(name="psA", bufs=3, space=PS) as psA,
            tc.tile_pool(name="psB", bufs=1, space=PS) as psB,
        ):
            # ---- resident loads ----
            xt = [cp.tile([128, BL], F32R, name=f"xt{k}", tag=f"xt{k}") for k in range(4)]
            for k in range(4):
                nc.sync.dma_start(xt[k][:], d["xT"][k * 128:(k + 1) * 128, :])
            epsT = cp.tile([S, BL], F32R)
            nc.sync.dma_start(epsT[:], d["epsT"][:])
            ew1 = cp.tile([128, 4 * H], F32R)
            nc.sync.dma_start(
                ew1[:].rearrange("k (g m) -> k g m", g=4),
                d["ew1"][:].rearrange("(g k) m -> k g m", g=4))
            ew2 = cp.tile([128, 4 * 2 * S], F32R)
            nc.sync.dma_start(
                ew2[:].rearrange("k (g m) -> k g m", g=4),
                d["ew2"][:].rearrange("(g k) m -> k g m", g=4))
            eb1 = cp.tile([128, 4], F32)
            nc.sync.dma_start(eb1[:], d["eb1"][:])
            eb2 = cp.tile([S, 3], F32)
            nc.sync.dma_start(eb2[:], d["eb2"][:])
            mb2 = cp.tile([S, 1], F32)
            nc.sync.dma_start(mb2[:], d["mb2"][:])
            dw1 = cp.tile([S, H], F32R)
            nc.sync.dma_start(dw1[:], d["dw1"][:])
            dw2 = cp.tile([128, 4 * F], F32R)
            nc.sync.dma_start(
                dw2[:].rearrange("k (g m) -> k g m", g=4),
                d["dw2"][:].rearrange("(g k) m -> k g m", g=4))
            db1 = cp.tile([128, 4], F32)
            nc.sync.dma_start(db1[:], d["db1"][:])
            db2 = cp.tile([128, 4], F32)
            nc.sync.dma_start(db2[:], d["db2"][:])
            m2wm = cp.tile([128, 2 * S * S], F32R)
            nc.sync.dma_start(m2wm[:], d["m2w"][:])
            b2p = cp.tile([128, NBLK], F32)
            nc.sync.dma_start(b2p[:], d["b2p"][:])
            sT = cp.tile([128, NBLK], F32)
            klp = cp.tile([S, 3], F32)
            rsq = cp.tile([128, 4], F32)
            zTe = cp.tile([S + 1, BL], F32R)

            # ---- encoder layer 1: hT = lrelu(W1^T xT + b1) ----
            with tc.tile_pool(name="encw", bufs=1) as ewp:
                m1wm = ewp.tile([S + 1, S * H2], F32R)
                nc.sync.dma_start(m1wm[:], d["m1w"][:])
                hT = [cp.tile([128, BL], F32R, name=f"hT{m}", tag=f"hT{m}") for m in range(4)]
                for mt in range(4):
                    p_ = psA.tile([128, BL], F32, tag="pt", bufs=2)
                    for kt in range(4):
                        nc.tensor.matmul(
                            p_[:], ew1[:, 4 * 128 * kt + mt * 128:
                                        4 * 128 * kt + (mt + 1) * 128]
                            .rearrange("k m -> k m"),
                            xt[kt][:], start=(kt == 0), stop=(kt == 3))
                    nc.scalar.activation(hT[mt][:], p_[:], AF.Prelu,
                                         bias=eb1[:, mt:mt + 1], alpha=LEAK)

                # ---- encoder layer 2: params = W2^T hT + b2 ----
                pp = psA.tile([2 * S, BL], F32, tag="pt", bufs=2)
                for kt in range(4):
                    nc.tensor.matmul(
                        pp[:], ew2[:, kt * 2 * S:(kt + 1) * 2 * S],
                        hT[kt][:], start=(kt == 0), stop=(kt == 3))
                meanT = cp.tile([S, BL], F32R)
                stdT = cp.tile([S, BL], F32R)
                nc.vector.tensor_scalar_add(meanT[:], pp[0:S, :], eb2[:, 0:1])
                nc.scalar.activation(stdT[:], pp[S:2 * S, :], AF.Exp,
                                     bias=eb2[:, 1:2])
                nc.sync.dma_start(d["meanT"][:], meanT[:])
                nc.sync.dma_start(d["stdT"][:], stdT[:])
                # kl partial sums (scratch outputs unused); eb2 col2 = 2*ls bias
                ksc = scp.tile([S, BL], F32, tag="ksc")
                nc.scalar.activation(ksc[:], pp[0:S, :], AF.Square,
                                     bias=eb2[:, 0:1], accum_out=klp[:, 0:1])
                ksc3 = scp.tile([S, BL], F32, tag="ksc")
                nc.scalar.activation(ksc3[:], pp[S:2 * S, :], AF.Exp, scale=2.0,
                                     bias=eb2[:, 2:3], accum_out=klp[:, 1:2])
                ksc4 = scp.tile([S, BL], F32, tag="ksc")
                nc.scalar.activation(ksc4[:], pp[S:2 * S, :], AF.Identity,
                                     bias=eb2[:, 1:2], accum_out=klp[:, 2:3])
                nc.sync.dma_start(d["klp"][:], klp[:])

                # ---- z = mean + std * eps ; append ones row ----
                nc.vector.tensor_tensor(zTe[0:S, :], stdT[:], epsT[:], ALU.mult)
                nc.vector.tensor_tensor(zTe[0:S, :], zTe[0:S, :], meanT[:], ALU.add)
                nc.vector.tensor_scalar(zTe[S:S + 1, :], epsT[0:1, :], 0.0, 1.0,
                                        ALU.mult, ALU.add)

                # ---- mech + MI: one interleaved two-stage pipeline ----
                # Unit = either one mech i (two 128-ch chunks) or one MI
                # double-block (two 2-pair blocks). Stage1 = matmuls into a
                # [128, 2*BL] psum + one leaky eviction (ACT Prelu or DVE
                # mul+max, round-robin for engine balance). Stage2 (emitted
                # DEPTH units behind its stream) = the consuming matmuls +
                # ACT evict. Interleaving keeps ACT/DVE fed from the start.
                DEPTH = 2
                NDBL = NBLK // 2
                zcp = psB.tile([S, BL], F32, tag="zcp")
                zTeb = cp.tile([S + 1, BL], F32R)
                h1ds, m1ds, g1s, g2s = {}, {}, {}, {}
                flexctr = [0]

                def leaky_evict(dst, srcp):
                    k = flexctr[0]
                    flexctr[0] += 1
                    if k % 10 < 4:
                        nc.scalar.activation(dst[:], srcp[:], AF.Prelu, alpha=LEAK)
                    else:
                        t0 = scp.tile([128, 2 * BL], F32R, tag="t0")
                        nc.vector.tensor_scalar_mul(t0[:], srcp[:], LEAK)
                        nc.vector.tensor_tensor(dst[:], srcp[:], t0[:], ALU.max)

                def mech_stage1(u):
                    hp = psA.tile([128, 2 * BL], F32, tag="pt", bufs=2)
                    for c in range(2):
                        nc.tensor.matmul(
                            hp[:, c * BL:(c + 1) * BL],
                            m1wm[:, u * H2 + c * 128: u * H2 + (c + 1) * 128],
                            zTe[:], start=True, stop=True)
                    h1d = m1p.tile([128, 2 * BL], F32R, tag="h1t", bufs=DEPTH + 2)
                    leaky_evict(h1d, hp)
                    h1ds[u] = h1d

                def mech_stage2(u):
                    h1d = h1ds.pop(u)
                    for c in range(2):
                        j = 2 * u + c
                        nc.tensor.matmul(
                            zcp[:], m2wm[:, j * S:(j + 1) * S],
                            h1d[:, c * BL:(c + 1) * BL],
                            start=(j == 0), stop=(j == 2 * S - 1))

                def mi_stage1(u):
                    for h in range(2):
                        b = 2 * u + h
                        g, j = divmod(b, GRP)
                        if j == 0:
                            g1 = g1p.tile([S + 1, GRP * 128], F32R, tag="g1")
                            nc.sync.dma_start(g1[:], d["mi1w"][g])
                            g2 = g2p.tile([128, GRP * 128], F32R, tag="g2")
                            nc.sync.dma_start(g2[:], d["mi2w"][g])
                            g1s[g] = g1
                            g2s[g] = g2
                    mp1 = psA.tile([128, 2 * BL], F32, tag="pt", bufs=2)
                    for h in range(2):
                        b = 2 * u + h
                        g, j = divmod(b, GRP)
                        nc.tensor.matmul(mp1[:, h * BL:(h + 1) * BL],
                                         g1s[g][:, j * 128:(j + 1) * 128],
                                         zTeb[:], start=True, stop=True)
                    m1d = m1p.tile([128, 2 * BL], F32R, tag="m1t", bufs=DEPTH + 2)
                    leaky_evict(m1d, mp1)
                    m1ds[u] = m1d

                def mi_stage2(u):
                    m1d = m1ds.pop(u)
                    for h in range(2):
                        b = 2 * u + h
                        g, j = divmod(b, GRP)
                        mp2 = psA.tile([128, BL], F32, tag="pt2", bufs=3)
                        nc.tensor.matmul(mp2[:], g2s[g][:, j * 128:(j + 1) * 128],
                                         m1d[:, h * BL:(h + 1) * BL],
                                         start=True, stop=True)
                        m2o = scp.tile([128, BL], F32, tag="m2o")
                        nc.scalar.activation(m2o[:], mp2[:], AF.Prelu,
                                             bias=b2p[:, b:b + 1], alpha=LEAK,
                                             accum_out=sT[:, b:b + 1])

                nc.vector.tensor_copy(zTeb[:], zTe[:])
                # interleave schedule: one mech unit every 5th slot
                sched = []
                mi_i = me_i = 0
                while mi_i < NDBL or me_i < S:
                    if (len(sched) % 5 == 4 and me_i < S) or mi_i >= NDBL:
                        sched.append(("mech", me_i)); me_i += 1
                    else:
                        sched.append(("mi", mi_i)); mi_i += 1
                done1 = []
                for kind, u in sched:
                    (mech_stage1 if kind == "mech" else mi_stage1)(u)
                    done1.append((kind, u))
                    if len(done1) > DEPTH:
                        k2, u2 = done1[len(done1) - 1 - DEPTH]
                        (mech_stage2 if k2 == "mech" else mi_stage2)(u2)
                for k2, u2 in done1[len(done1) - DEPTH:]:
                    (mech_stage2 if k2 == "mech" else mi_stage2)(u2)
                nc.sync.dma_start(d["sT"][:], sT[:])

                zcT = cp.tile([S, BL], F32R)
                nc.vector.tensor_scalar_add(zcT[:], zcp[:], mb2[:])
                nc.sync.dma_start(d["zcT"][:], zcT[:])

            # ---- decoder ----
            d1T = [cp.tile([128, BL], F32R, name=f"d1T{m}", tag=f"d1T{m}") for m in range(4)]
            for mt in range(4):
                p_ = psA.tile([128, BL], F32, tag="pt", bufs=2)
                nc.tensor.matmul(p_[:], dw1[:, mt * 128:(mt + 1) * 128],
                                 zcT[:], start=True, stop=True)
                nc.scalar.activation(d1T[mt][:], p_[:], AF.Prelu,
                                     bias=db1[:, mt:mt + 1], alpha=LEAK)
            for mt in range(4):
                p_ = psA.tile([128, BL], F32, tag="pt", bufs=2)
                for kt in range(4):
                    nc.tensor.matmul(
                        p_[:], dw2[:, 4 * 128 * kt + mt * 128:
                                    4 * 128 * kt + (mt + 1) * 128]
                        .rearrange("k m -> k m"),
                        d1T[kt][:], start=(kt == 0), stop=(kt == 3))
                rt = scp.tile([128, BL], F32R, tag="rt")
                nc.vector.tensor_scalar_add(rt[:], p_[:], db2[:, mt:mt + 1])
                nc.sync.dma_start(d["reconT"][mt * 128:(mt + 1) * 128, :], rt[:])
                df = scp.tile([128, BL], F32R, tag="df")
                nc.vector.tensor_tensor(df[:], rt[:], xt[mt][:], ALU.subtract)
                dsc = scp.tile([128, BL], F32, tag="dsc")
                nc.scalar.activation(dsc[:], df[:], AF.Square,
                                     accum_out=rsq[:, mt:mt + 1])
            nc.sync.dma_start(d["rsq"][:], rsq[:])

    nc.compile()
    return nc


def _pack_host(inputs):
    """Build per-core input maps from full inputs."""
    f32 = np.float32
    g = {k: np.asarray(v, dtype=f32) for k, v in inputs.items()}
    lw = g["log_weight"]
    eye = np.eye(S, dtype=f32)
    upper = np.triu(np.ones((S, S), f32), k=1)
    x_ = (lw * upper - 10.0 * eye) / TEMP
    A = np.where(
        x_ >= 0, 1.0 / (1.0 + np.exp(-np.clip(x_, 0, None))),
        np.exp(np.clip(x_, None, 0)) / (1.0 + np.exp(np.clip(x_, None, 0)))
    ).astype(f32)

    # mech layer-1 lhsT with A absorbed + bias row: [S+1, S*H2]
    w1a = np.einsum("si,ish->ish", A, g["mech_w1"])  # w~1[i,s,h] = A[s,i]*w1[i,s,h]
    m1w = np.zeros((S + 1, S * H2), f32)
    m1w[0:S] = w1a.transpose(1, 0, 2).reshape(S, S * H2)
    m1w[S] = g["mech_b1"].reshape(S * H2)
    # mech layer-2 lhsT: [128, 2*S*S]; col block j=(i,c): column i = w2[i, c*128:...]
    m2w = np.zeros((128, 2 * S, S), f32)
    ii = np.arange(S)
    for c in range(2):
        m2w[:, 2 * ii + c, ii] = g["mech_w2"][:, c * 128:(c + 1) * 128].T
    m2w = m2w.reshape(128, 2 * S * S)

    iu, ju = np.triu_indices(S, k=1)
    # MI layer-1 sparse lhsT groups [NGRP, S+1, GRP*128]
    mi1 = np.zeros((NBLK, S + 1, 128), f32)
    pw = g["mi_w1"]  # [P, 2, CH]
    pb = g["mi_b1"]  # [P, CH]
    bi = np.arange(NBLK)[:, None]
    ci = np.arange(CH)[None, :]
    for h in range(2):
        p_idx = 2 * np.arange(NBLK) + h
        cols = slice(h * CH, (h + 1) * CH)
        mi1[bi, iu[p_idx][:, None], ci + h * CH] = pw[p_idx, 0]
        mi1[bi, ju[p_idx][:, None], ci + h * CH] += pw[p_idx, 1]
        mi1[:, S, cols] = pb[p_idx]
    mi1w = mi1.reshape(NGRP, GRP, S + 1, 128).transpose(0, 2, 1, 3) \
              .reshape(NGRP, S + 1, GRP * 128).copy()
    # MI layer-2 block-diag groups [NGRP, 128, GRP*128]
    mi2 = np.zeros((NBLK, 128, 128), f32)
    mi2[:, 0:CH, 0:CH] = g["mi_w2"][0::2]
    mi2[:, CH:128, CH:128] = g["mi_w2"][1::2]
    mi2w = mi2.reshape(NGRP, GRP, 128, 128).transpose(0, 2, 1, 3) \
              .reshape(NGRP, 128, GRP * 128).copy()
    b2p = np.concatenate([g["mi_b2"][0::2], g["mi_b2"][1::2]], axis=1).T.copy()
    # ^ [128, NBLK]: rows 0:64 pair 2b bias, 64:128 pair 2b+1

    eb2 = np.stack([g["enc_b2"][0:S], g["enc_b2"][S:2 * S],
                    2.0 * g["enc_b2"][S:2 * S]], axis=1)

    def pack_bias_cols(bvec):  # [512] -> [128, 4]
        return bvec.reshape(4, 128).T.copy()

    shared = {
        "ew1": g["enc_w1"], "ew2": g["enc_w2"],
        "eb1": pack_bias_cols(g["enc_b1"]), "eb2": eb2,
        "m1w": m1w, "m2w": m2w, "mb2": g["mech_b2"][:, None],
        "dw1": g["dec_w1"], "dw2": g["dec_w2"],
        "db1": pack_bias_cols(g["dec_b1"]), "db2": pack_bias_cols(g["dec_b2"]),
        "mi1w": mi1w, "mi2w": mi2w, "b2p": b2p,
    }
    shared = {k: np.ascontiguousarray(v, dtype=f32) for k, v in shared.items()}

    in_maps = []
    for c in range(NCORES):
        sl = slice(c * BL, (c + 1) * BL)
        m = dict(shared)
        m["xT"] = np.ascontiguousarray(g["features"][sl].T)
        m["epsT"] = np.ascontiguousarray(g["eps"][sl].T)
        in_maps.append(m)
    return in_maps, g, A, (iu, ju)


def kernel(**inputs):
    if "nc" not in _CACHE:
        _CACHE["nc"] = _build_nc()
    nc = _CACHE["nc"]

    in_maps, g, A, (iu, ju) = _pack_host(inputs)
    res = run_bass_kernel_spmd(nc, in_maps, list(range(NCORES))).results

    f32 = np.float32
    mean = np.concatenate([r["meanT"].T for r in res], axis=0)
    std = np.concatenate([r["stdT"].T for r in res], axis=0)
    zc = np.concatenate([r["zcT"].T for r in res], axis=0)
    recon = np.concatenate([r["reconT"].T for r in res], axis=0)

    klp = sum(r["klp"] for r in res)          # [S, 3]
    rsq_tot = float(sum(r["rsq"].sum() for r in res))
    sT = sum(r["sT"].astype(np.float64) for r in res)  # [128, NBLK]

    recon_loss = rsq_tot / (B * F)
    kl_loss = 0.5 * (klp[:, 0].sum() + klp[:, 1].sum()
                     - 2.0 * klp[:, 2].sum() - B * S) / B

    lw = g["log_weight"]
    sig = 1.0 / (1.0 + np.exp(-lw))
    sparsity_loss = float(np.abs(sig).sum())

    # mi_est[p] = (w3[p] . sum_b m2[p]) / B + b3[p]
    s_pairs = np.empty((P, CH), np.float64)
    s_pairs[0::2] = sT[0:CH].T
    s_pairs[1::2] = sT[CH:128].T
    mi_est = (s_pairs * g["mi_w3"]).sum(axis=1) / B + g["mi_b3"]
    mi_loss = float((sig[iu, ju] * mi_est).sum())

    A2 = (sig * sig).astype(f32)
    expm = np.eye(S, dtype=f32)
    mp = np.eye(S, dtype=f32)
    fact = 1.0
    for i in range(1, 10):
        fact *= i
        mp = mp @ A2
        expm = expm + mp / fact
    dag_loss = float(np.trace(expm) - S) ** 2

    total = recon_loss + kl_loss + 0.1 * sparsity_loss + 0.01 * mi_loss + 1.0 * dag_loss

    def sc(x):
        return np.float32(x)

    return (zc.astype(f32), mean.astype(f32), std.astype(f32), recon.astype(f32),
            sc(total), sc(recon_loss), sc(kl_loss), sc(sparsity_loss),
            sc(mi_loss), sc(dag_loss))


# revision 23
# speedup vs baseline: 1.0129x; 1.0129x over previous
"""Trainium2 Bass kernel for nn_CausalRSSM: data-parallel over batch on 8 cores.

Layout: all activations transposed (feature dim on SBUF partitions, batch on
free dim), so every layer is matmul(lhsT=W[K,M], rhs=X^T[K,B]) with weights in
their natural [in, out] storage order. Host pre-transposes features/eps,
absorbs the adjacency A into mech_w1, packs the 496 MI pair-MLPs into
2-pairs-per-matmul blocks (sparse lhsT rows vs the shared z rhsm, block-diag
second layer), and finishes scalar loss reductions from per-core partial sums.
"""
import sys
import numpy as np

for _p in ("/opt/trn_rl_repo",):
    if _p not in sys.path:
        sys.path.append(_p)

import concourse.bass as bass
import concourse.mybir as mybir
import concourse.tile as tile
from concourse import bacc
from concourse.bass_utils import run_bass_kernel_spmd

F32 = mybir.dt.float32
F32R = mybir.dt.float32r
AF = mybir.ActivationFunctionType
ALU = mybir.AluOpType
PS = bass.MemorySpace.PSUM

# problem dims (hardcoded per spec)
B, F, S, H = 4096, 512, 32, 512
H2, CH = 256, 64
P = S * (S - 1) // 2          # 496 pairs
NCORES = 8
BL = B // NCORES              # 512 rows per core
NBLK = P // 2                 # 248 two-pair blocks
GRP = 8                       # MI blocks per streamed weight group
NGRP = NBLK // GRP            # 31
LEAK = 0.2
TEMP = 0.1

_CACHE = {}


def _build_nc():
    nc = bacc.Bacc(None)
    d = {}
    def din(name, shape, dt=F32R):
        d[name] = nc.declare_dram_parameter(name, list(shape), dt, isOutput=False)
    def dout(name, shape, dt=F32R):
        d[name] = nc.declare_dram_parameter(name, list(shape), dt, isOutput=True)

    din("xT", (F, BL))              # features^T
    din("epsT", (S, BL))
    din("ew1", (F, H))              # enc_w1 [K,M]
    din("ew2", (F, 2 * S))
    din("eb1", (128, 4), F32)       # enc_b1 packed per M-tile column
    din("eb2", (S, 3), F32)         # cols: mean bias, ls bias, 2*ls bias
    din("m1w", (S + 1, S * H2))     # mech layer1 lhsT (+bias row), col blk (i,c)
    din("m2w", (128, 2 * S * S))    # mech layer2 lhsT, col blk (i,c) x 32 cols
    din("mb2", (S, 1), F32)         # mech_b2
    din("dw1", (S, H))
    din("dw2", (H, F))
    din("db1", (128, 4), F32)
    din("db2", (128, 4), F32)
    din("mi1w", (NGRP, S + 1, GRP * 128))   # MI layer1 lhsT groups (+bias row)
    din("mi2w", (NGRP, 128, GRP * 128))     # MI layer2 block-diag lhsT groups
    din("b2p", (128, NBLK), F32)            # mi_b2 packed 2 pairs per col

    dout("meanT", (S, BL))
    dout("stdT", (S, BL))
    dout("zcT", (S, BL))
    dout("reconT", (F, BL))
    dout("sT", (128, NBLK), F32)    # sum_b lrelu(m2) per (pair-of-2, ch)
    dout("klp", (S, 3), F32)        # sum mean^2, sum std^2, sum log_std
    dout("rsq", (128, 4), F32)      # sum (recon - x)^2 per dec2 M-tile

    with tile.TileContext(nc) as tc:
        with (
            tc.tile_pool(name="cst", bufs=1) as cp,
            tc.tile_pool(name="g1", bufs=3) as g1p,
            tc.tile_pool(name="g2", bufs=3) as g2p,
            tc.tile_pool(name="m1t", bufs=4) as m1p,
            tc.tile_pool(name="scr", bufs=3) as scp,
            tc.tile_pool(name="psA", bufs=3, space=PS) as psA,
            tc.tile_pool(name="psB", bufs=1, space=PS) as psB,
        ):
            # ---- resident loads ----
            xt = [cp.tile([128, BL], F32R, name=f"xt{k}", tag=f"xt{k}") for k in range(4)]
            for k in range(4):
                nc.sync.dma_start(xt[k][:], d["xT"][k * 128:(k + 1) * 128, :])
            epsT = cp.tile([S, BL], F32R)
            nc.sync.dma_start(epsT[:], d["epsT"][:])
            ew1 = cp.tile([128, 4 * H], F32R)
            nc.sync.dma_start(
                ew1[:].rearrange("k (g m) -> k g m", g=4),
                d["ew1"][:].rearrange("(g k) m -> k g m", g=4))
            ew2 = cp.tile([128, 4 * 2 * S], F32R)
            nc.sync.dma_start(
                ew2[:].rearrange("k (g m) -> k g m", g=4),
                d["ew2"][:].rearrange("(g k) m -> k g m", g=4))
            eb1 = cp.tile([128, 4], F32)
            nc.sync.dma_start(eb1[:], d["eb1"][:])
            eb2 = cp.tile([S, 3], F32)
            nc.sync.dma_start(eb2[:], d["eb2"][:])
            mb2 = cp.tile([S, 1], F32)
            nc.sync.dma_start(mb2[:], d["mb2"][:])
            dw1 = cp.tile([S, H], F32R)
            nc.sync.dma_start(dw1[:], d["dw1"][:])
            dw2 = cp.tile([128, 4 * F], F32R)
            nc.sync.dma_start(
                dw2[:].rearrange("k (g m) -> k g m", g=4),
                d["dw2"][:].rearrange("(g k) m -> k g m", g=4))
            db1 = cp.tile([128, 4], F32)
            nc.sync.dma_start(db1[:], d["db1"][:])
            db2 = cp.tile([128, 4], F32)
            nc.sync.dma_start(db2[:], d["db2"][:])
            m2wm = cp.tile([128, 2 * S * S], F32R)
            nc.sync.dma_start(m2wm[:], d["m2w"][:])
            b2p = cp.tile([128, NBLK], F32)
            nc.sync.dma_start(b2p[:], d["b2p"][:])
            sT = cp.tile([128, NBLK], F32)
            klp = cp.tile([S, 3], F32)
            rsq = cp.tile([128, 4], F32)
            zTe = cp.tile([S + 1, BL], F32R)

            # ---- encoder layer 1: hT = lrelu(W1^T xT + b1) ----
            with tc.tile_pool(name="encw", bufs=1) as ewp:
                m1wm = ewp.tile([S + 1, S * H2], F32R)
                nc.sync.dma_start(m1wm[:], d["m1w"][:])
                hT = [cp.tile([128, BL], F32R, name=f"hT{m}", tag=f"hT{m}") for m in range(4)]
                for mt in range(4):
                    p_ = psA.tile([128, BL], F32, tag="pt", bufs=2)
                    for kt in range(4):
                        nc.tensor.matmul(
                            p_[:], ew1[:, 4 * 128 * kt + mt * 128:
                                        4 * 128 * kt + (mt + 1) * 128]
                            .rearrange("k m -> k m"),
                            xt[kt][:], start=(kt == 0), stop=(kt == 3))
                    nc.scalar.activation(hT[mt][:], p_[:], AF.Prelu,
                                         bias=eb1[:, mt:mt + 1], alpha=LEAK)

                # ---- encoder layer 2: params = W2^T hT + b2 ----
                pp = psA.tile([2 * S, BL], F32, tag="pt", bufs=2)
                for kt in range(4):
                    nc.tensor.matmul(
                        pp[:], ew2[:, kt * 2 * S:(kt + 1) * 2 * S],
                        hT[kt][:], start=(kt == 0), stop=(kt == 3))
                meanT = cp.tile([S, BL], F32R)
                stdT = cp.tile([S, BL], F32R)
                nc.vector.tensor_scalar_add(meanT[:], pp[0:S, :], eb2[:, 0:1])
                nc.scalar.activation(stdT[:], pp[S:2 * S, :], AF.Exp,
                                     bias=eb2[:, 1:2])
                nc.sync.dma_start(d["meanT"][:], meanT[:])
                nc.sync.dma_start(d["stdT"][:], stdT[:])
                # kl partial sums (scratch outputs unused); eb2 col2 = 2*ls bias
                ksc = scp.tile([S, BL], F32, tag="ksc")
                nc.scalar.activation(ksc[:], pp[0:S, :], AF.Square,
                                     bias=eb2[:, 0:1], accum_out=klp[:, 0:1])
                ksc3 = scp.tile([S, BL], F32, tag="ksc")
                nc.scalar.activation(ksc3[:], pp[S:2 * S, :], AF.Exp, scale=2.0,
                                     bias=eb2[:, 2:3], accum_out=klp[:, 1:2])
                ksc4 = scp.tile([S, BL], F32, tag="ksc")
                nc.scalar.activation(ksc4[:], pp[S:2 * S, :], AF.Identity,
                                     bias=eb2[:, 1:2], accum_out=klp[:, 2:3])
                nc.sync.dma_start(d["klp"][:], klp[:])

                # ---- z = mean + std * eps ; append ones row ----
                nc.vector.tensor_tensor(zTe[0:S, :], stdT[:], epsT[:], ALU.mult)
                nc.vector.tensor_tensor(zTe[0:S, :], zTe[0:S, :], meanT[:], ALU.add)
                nc.vector.tensor_scalar(zTe[S:S + 1, :], epsT[0:1, :], 0.0, 1.0,
                                        ALU.mult, ALU.add)

                # ---- mech + MI: one interleaved two-stage pipeline ----
                # Unit = either one mech i (two 128-ch chunks) or one MI
                # double-block (two 2-pair blocks). Stage1 = matmuls into a
                # [128, 2*BL] psum + one leaky eviction (ACT Prelu or DVE
                # mul+max, round-robin for engine balance). Stage2 (emitted
                # DEPTH units behind its stream) = the consuming matmuls +
                # ACT evict. Interleaving keeps ACT/DVE fed from the start.
                DEPTH = 2
                NDBL = NBLK // 2
                zcp = psB.tile([S, BL], F32, tag="zcp")
                zTeb = cp.tile([S + 1, BL], F32R)
                h1ds, m1ds, g1s, g2s = {}, {}, {}, {}
                flexctr = [0]

                def leaky_evict(dst, srcp):
                    k = flexctr[0]
                    flexctr[0] += 1
                    if k % 10 < 4:
                        nc.scalar.activation(dst[:], srcp[:], AF.Prelu, alpha=LEAK)
                    else:
                        t0 = scp.tile([128, 2 * BL], F32R, tag="t0")
                        nc.vector.tensor_scalar_mul(t0[:], srcp[:], LEAK)
                        nc.vector.tensor_tensor(dst[:], srcp[:], t0[:], ALU.max)

                def mech_stage1(u):
                    hp = psA.tile([128, 2 * BL], F32, tag="pt", bufs=2)
                    for c in range(2):
                        nc.tensor.matmul(
                            hp[:, c * BL:(c + 1) * BL],
                            m1wm[:, u * H2 + c * 128: u * H2 + (c + 1) * 128],
                            zTe[:], start=True, stop=True)
                    h1d = m1p.tile([128, 2 * BL], F32R, tag="h1t", bufs=DEPTH + 2)
                    leaky_evict(h1d, hp)
                    h1ds[u] = h1d

                def mech_stage2(u):
                    h1d = h1ds.pop(u)
                    for c in range(2):
                        j = 2 * u + c
                        nc.tensor.matmul(
                            zcp[:], m2wm[:, j * S:(j + 1) * S],
                            h1d[:, c * BL:(c + 1) * BL],
                            start=(j == 0), stop=(j == 2 * S - 1))

                def mi_stage1(u):
                    for h in range(2):
                        b = 2 * u + h
                        g, j = divmod(b, GRP)
                        if j == 0:
                            g1 = g1p.tile([S + 1, GRP * 128], F32R, tag="g1")
                            nc.sync.dma_start(g1[:], d["mi1w"][g])
                            g2 = g2p.tile([128, GRP * 128], F32R, tag="g2")
                            nc.sync.dma_start(g2[:], d["mi2w"][g])
                            g1s[g] = g1
                            g2s[g] = g2
                    mp1 = psA.tile([128, 2 * BL], F32, tag="pt", bufs=2)
                    for h in range(2):
                        b = 2 * u + h
                        g, j = divmod(b, GRP)
                        nc.tensor.matmul(mp1[:, h * BL:(h + 1) * BL],
                                         g1s[g][:, j * 128:(j + 1) * 128],
                                         zTeb[:], start=True, stop=True)
                    m1d = m1p.tile([128, 2 * BL], F32R, tag="m1t", bufs=DEPTH + 2)
                    leaky_evict(m1d, mp1)
                    m1ds[u] = m1d

                def mi_stage2(u):
                    m1d = m1ds.pop(u)
                    for h in range(2):
                        b = 2 * u + h
                        g, j = divmod(b, GRP)
                        mp2 = psA.tile([128, BL], F32, tag="pt2", bufs=3)
                        nc.tensor.matmul(mp2[:], g2s[g][:, j * 128:(j + 1) * 128],
                                         m1d[:, h * BL:(h + 1) * BL],
                                         start=True, stop=True)
                        nc.scalar.activation(mp2[:], mp2[:], AF.Prelu,
                                             bias=b2p[:, b:b + 1], alpha=LEAK,
                                             accum_out=sT[:, b:b + 1])

                nc.vector.tensor_copy(zTeb[:], zTe[:])
                # interleave schedule: one mech unit every 5th slot
                sched = []
                mi_i = me_i = 0
                while mi_i < NDBL or me_i < S:
                    if (len(sched) % 5 == 4 and me_i < S) or mi_i >= NDBL:
                        sched.append(("mech", me_i)); me_i += 1
                    else:
                        sched.append(("mi", mi_i)); mi_i += 1
                done1 = []
                for kind, u in sched:
                    (mech_stage1 if kind == "mech" else mi_stage1)(u)
                    done1.append((kind, u))
                    if len(done1) > DEPTH:
                        k2, u2 = done1[len(done1) - 1 - DEPTH]
                        (mech_stage2 if k2 == "mech" else mi_stage2)(u2)
                for k2, u2 in done1[len(done1) - DEPTH:]:
                    (mech_stage2 if k2 == "mech" else mi_stage2)(u2)
                nc.sync.dma_start(d["sT"][:], sT[:])

                zcT = cp.tile([S, BL], F32R)
                nc.vector.tensor_scalar_add(zcT[:], zcp[:], mb2[:])
                nc.sync.dma_start(d["zcT"][:], zcT[:])

            # ---- decoder ----
            d1T = [cp.tile([128, BL], F32R, name=f"d1T{m}", tag=f"d1T{m}") for m in range(4)]
            for mt in range(4):
                p_ = psA.tile([128, BL], F32, tag="pt", bufs=2)
                nc.tensor.matmul(p_[:], dw1[:, mt * 128:(mt + 1) * 128],
                                 zcT[:], start=True, stop=True)
                nc.scalar.activation(d1T[mt][:], p_[:], AF.Prelu,
                                     bias=db1[:, mt:mt + 1], alpha=LEAK)
            for mt in range(4):
                p_ = psA.tile([128, BL], F32, tag="pt", bufs=2)
                for kt in range(4):
                    nc.tensor.matmul(
                        p_[:], dw2[:, 4 * 128 * kt + mt * 128:
                                    4 * 128 * kt + (mt + 1) * 128]
                        .rearrange("k m -> k m"),
                        d1T[kt][:], start=(kt == 0), stop=(kt == 3))
                rt = scp.tile([128, BL], F32R, tag="rt")
                nc.vector.tensor_scalar_add(rt[:], p_[:], db2[:, mt:mt + 1])
                nc.sync.dma_start(d["reconT"][mt * 128:(mt + 1) * 128, :], rt[:])
                df = scp.tile([128, BL], F32R, tag="df")
                nc.vector.tensor_tensor(df[:], rt[:], xt[mt][:], ALU.subtract)
                dsc = scp.tile([128, BL], F32, tag="dsc")
                nc.scalar.activation(dsc[:], df[:], AF.Square,
                                     accum_out=rsq[:, mt:mt + 1])
            nc.sync.dma_start(d["rsq"][:], rsq[:])

    nc.compile()
    return nc


def _pack_host(inputs):
    """Build per-core input maps from full inputs."""
    f32 = np.float32
    g = {k: np.asarray(v, dtype=f32) for k, v in inputs.items()}
    lw = g["log_weight"]
    eye = np.eye(S, dtype=f32)
    upper = np.triu(np.ones((S, S), f32), k=1)
    x_ = (lw * upper - 10.0 * eye) / TEMP
    A = np.where(
        x_ >= 0, 1.0 / (1.0 + np.exp(-np.clip(x_, 0, None))),
        np.exp(np.clip(x_, None, 0)) / (1.0 + np.exp(np.clip(x_, None, 0)))
    ).astype(f32)

    # mech layer-1 lhsT with A absorbed + bias row: [S+1, S*H2]
    w1a = np.einsum("si,ish->ish", A, g["mech_w1"])  # w~1[i,s,h] = A[s,i]*w1[i,s,h]
    m1w = np.zeros((S + 1, S * H2), f32)
    m1w[0:S] = w1a.transpose(1, 0, 2).reshape(S, S * H2)
    m1w[S] = g["mech_b1"].reshape(S * H2)
    # mech layer-2 lhsT: [128, 2*S*S]; col block j=(i,c): column i = w2[i, c*128:...]
    m2w = np.zeros((128, 2 * S, S), f32)
    ii = np.arange(S)
    for c in range(2):
        m2w[:, 2 * ii + c, ii] = g["mech_w2"][:, c * 128:(c + 1) * 128].T
    m2w = m2w.reshape(128, 2 * S * S)

    iu, ju = np.triu_indices(S, k=1)
    # MI layer-1 sparse lhsT groups [NGRP, S+1, GRP*128]
    mi1 = np.zeros((NBLK, S + 1, 128), f32)
    pw = g["mi_w1"]  # [P, 2, CH]
    pb = g["mi_b1"]  # [P, CH]
    bi = np.arange(NBLK)[:, None]
    ci = np.arange(CH)[None, :]
    for h in range(2):
        p_idx = 2 * np.arange(NBLK) + h
        cols = slice(h * CH, (h + 1) * CH)
        mi1[bi, iu[p_idx][:, None], ci + h * CH] = pw[p_idx, 0]
        mi1[bi, ju[p_idx][:, None], ci + h * CH] += pw[p_idx, 1]
        mi1[:, S, cols] = pb[p_idx]
    mi1w = mi1.reshape(NGRP, GRP, S + 1, 128).transpose(0, 2, 1, 3) \
              .reshape(NGRP, S + 1, GRP * 128).copy()
    # MI layer-2 block-diag groups [NGRP, 128, GRP*128]
    mi2 = np.zeros((NBLK, 128, 128), f32)
    mi2[:, 0:CH, 0:CH] = g["mi_w2"][0::2]
    mi2[:, CH:128, CH:128] = g["mi_w2"][1::2]
    mi2w = mi2.reshape(NGRP, GRP, 128, 128).transpose(0, 2, 1, 3) \
              .reshape(NGRP, 128, GRP * 128).copy()
    b2p = np.concatenate([g["mi_b2"][0::2], g["mi_b2"][1::2]], axis=1).T.copy()
    # ^ [128, NBLK]: rows 0:64 pair 2b bias, 64:128 pair 2b+1

    eb2 = np.stack([g["enc_b2"][0:S], g["enc_b2"][S:2 * S],
                    2.0 * g["enc_b2"][S:2 * S]], axis=1)

    def pack_bias_cols(bvec):  # [512] -> [128, 4]
        return bvec.reshape(4, 128).T.copy()

    shared = {
        "ew1": g["enc_w1"], "ew2": g["enc_w2"],
        "eb1": pack_bias_cols(g["enc_b1"]), "eb2": eb2,
        "m1w": m1w, "m2w": m2w, "mb2": g["mech_b2"][:, None],
        "dw1": g["dec_w1"], "dw2": g["dec_w2"],
        "db1": pack_bias_cols(g["dec_b1"]), "db2": pack_bias_cols(g["dec_b2"]),
        "mi1w": mi1w, "mi2w": mi2w, "b2p": b2p,
    }
    shared = {k: np.ascontiguousarray(v, dtype=f32) for k, v in shared.items()}

    in_maps = []
    for c in range(NCORES):
        sl = slice(c * BL, (c + 1) * BL)
        m = dict(shared)
        m["xT"] = np.ascontiguousarray(g["features"][sl].T)
        m["epsT"] = np.ascontiguousarray(g["eps"][sl].T)
        in_maps.append(m)
    return in_maps, g, A, (iu, ju)


def kernel(**inputs):
    if "nc" not in _CACHE:
        _CACHE["nc"] = _build_nc()
    nc = _CACHE["nc"]

    in_maps, g, A, (iu, ju) = _pack_host(inputs)
    res = run_bass_kernel_spmd(nc, in_maps, list(range(NCORES))).results

    f32 = np.float32
    mean = np.concatenate([r["meanT"].T for r in res], axis=0)
    std = np.concatenate([r["stdT"].T for r in res], axis=0)
    zc = np.concatenate([r["zcT"].T for r in res], axis=0)
    recon = np.concatenate([r["reconT"].T for r in res], axis=0)

    klp = sum(r["klp"] for r in res)          # [S, 3]
    rsq_tot = float(sum(r["rsq"].sum() for r in res))
    sT = sum(r["sT"].astype(np.float64) for r in res)  # [128, NBLK]

    recon_loss = rsq_tot / (B * F)
    kl_loss = 0.5 * (klp[:, 0].sum() + klp[:, 1].sum()
                     - 2.0 * klp[:, 2].sum() - B * S) / B

    lw = g["log_weight"]
    sig = 1.0 / (1.0 + np.exp(-lw))
    sparsity_loss = float(np.abs(sig).sum())

    # mi_est[p] = (w3[p] . sum_b m2[p]) / B + b3[p]
    s_pairs = np.empty((P, CH), np.float64)
    s_pairs[0::2] = sT[0:CH].T
    s_pairs[1::2] = sT[CH:128].T
    mi_est = (s_pairs * g["mi_w3"]).sum(axis=1) / B + g["mi_b3"]
    mi_loss = float((sig[iu, ju] * mi_est).sum())

    A2 = (sig * sig).astype(f32)
    expm = np.eye(S, dtype=f32)
    mp = np.eye(S, dtype=f32)
    fact = 1.0
    for i in range(1, 10):
        fact *= i
        mp = mp @ A2
        expm = expm + mp / fact
    dag_loss = float(np.trace(expm) - S) ** 2

    total = recon_loss + kl_loss + 0.1 * sparsity_loss + 0.01 * mi_loss + 1.0 * dag_loss

    def sc(x):
        return np.float32(x)

    return (zc.astype(f32), mean.astype(f32), std.astype(f32), recon.astype(f32),
            sc(total), sc(recon_loss), sc(kl_loss), sc(sparsity_loss),
            sc(mi_loss), sc(dag_loss))


# revision 28
# speedup vs baseline: 1.4893x; 1.4703x over previous
"""Trainium2 Bass kernel for nn_CausalRSSM: data-parallel over batch on 8 cores.

Layout: all activations transposed (feature dim on SBUF partitions, batch on
free dim), so every layer is matmul(lhsT=W[K,M], rhs=X^T[K,B]) with weights in
their natural [in, out] storage order. Host pre-transposes features/eps,
absorbs the adjacency A into mech_w1, packs the 496 MI pair-MLPs into
2-pairs-per-matmul blocks (sparse lhsT rows vs the shared z rhsm, block-diag
second layer), and finishes scalar loss reductions from per-core partial sums.
"""
import sys
import numpy as np

for _p in ("/opt/trn_rl_repo",):
    if _p not in sys.path:
        sys.path.append(_p)

import concourse.bass as bass
import concourse.mybir as mybir
import concourse.tile as tile
from concourse import bacc
from concourse.bass_utils import run_bass_kernel_spmd

F32 = mybir.dt.float32
F32R = mybir.dt.float32r
AF = mybir.ActivationFunctionType
ALU = mybir.AluOpType
PS = bass.MemorySpace.PSUM

# problem dims (hardcoded per spec)
B, F, S, H = 4096, 512, 32, 512
H2, CH = 256, 64
P = S * (S - 1) // 2          # 496 pairs
NCORES = 8
BL = B // NCORES              # 512 rows per core
NBLK = P // 2                 # 248 two-pair blocks
GRP = 8                       # MI blocks per streamed weight group
NGRP = NBLK // GRP            # 31
LEAK = 0.2
TEMP = 0.1

_CACHE = {}


def _build_nc():
    nc = bacc.Bacc(None)
    d = {}
    def din(name, shape, dt=F32R):
        d[name] = nc.declare_dram_parameter(name, list(shape), dt, isOutput=False)
    def dout(name, shape, dt=F32R):
        d[name] = nc.declare_dram_parameter(name, list(shape), dt, isOutput=True)

    din("xT", (F, BL))              # features^T
    din("epsT", (S, BL))
    din("ew1", (F, H))              # enc_w1 [K,M]
    din("ew2", (F, 2 * S))
    din("eb1", (128, 4), F32)       # enc_b1 packed per M-tile column
    din("eb2", (S, 3), F32)         # cols: mean bias, ls bias, 2*ls bias
    din("m1w", (S + 1, S * H2))     # mech layer1 lhsT (+bias row), col blk (i,c)
    din("m2w", (128, 2 * S * S))    # mech layer2 lhsT, col blk (i,c) x 32 cols
    din("mb2", (S, 1), F32)         # mech_b2
    din("dw1", (S, H))
    din("dw2", (H, F))
    din("db1", (128, 4), F32)
    din("db2", (128, 4), F32)
    din("mi1w", (NGRP, S + 1, GRP * 128))   # MI layer1 lhsT groups (+bias row)
    din("mi2w", (NGRP, 128, GRP * 128))     # MI layer2 block-diag lhsT groups
    din("b2p", (128, NBLK), F32)            # mi_b2 packed 2 pairs per col

    dout("meanT", (S, BL))
    dout("stdT", (S, BL))
    dout("zcT", (S, BL))
    dout("reconT", (F, BL))
    dout("sT", (128, NBLK), F32)    # sum_b lrelu(m2) per (pair-of-2, ch)
    dout("klp", (S, 3), F32)        # sum mean^2, sum std^2, sum log_std
    dout("rsq", (128, 4), F32)      # sum (recon - x)^2 per dec2 M-tile

    with tile.TileContext(nc) as tc:
        with (
            tc.tile_pool(name="cst", bufs=1) as cp,
            tc.tile_pool(name="g1", bufs=3) as g1p,
            tc.tile_pool(name="g2", bufs=3) as g2p,
            tc.tile_pool(name="m1t", bufs=4) as m1p,
            tc.tile_pool(name="scr", bufs=3) as scp,
            tc.tile_pool(name="psA", bufs=3, space=PS) as psA,
            tc.tile_pool(name="psB", bufs=1, space=PS) as psB,
        ):
            # ---- resident loads ----
            xt = [cp.tile([128, BL], F32R, name=f"xt{k}", tag=f"xt{k}") for k in range(4)]
            for k in range(4):
                nc.sync.dma_start(xt[k][:], d["xT"][k * 128:(k + 1) * 128, :])
            epsT = cp.tile([S, BL], F32R)
            nc.sync.dma_start(epsT[:], d["epsT"][:])
            ew1 = cp.tile([128, 4 * H], F32R)
            nc.sync.dma_start(
                ew1[:].rearrange("k (g m) -> k g m", g=4),
                d["ew1"][:].rearrange("(g k) m -> k g m", g=4))
            ew2 = cp.tile([128, 4 * 2 * S], F32R)
            nc.sync.dma_start(
                ew2[:].rearrange("k (g m) -> k g m", g=4),
                d["ew2"][:].rearrange("(g k) m -> k g m", g=4))
            eb1 = cp.tile([128, 4], F32)
            nc.sync.dma_start(eb1[:], d["eb1"][:])
            eb2 = cp.tile([S, 3], F32)
            nc.sync.dma_start(eb2[:], d["eb2"][:])
            mb2 = cp.tile([S, 1], F32)
            nc.sync.dma_start(mb2[:], d["mb2"][:])
            dw1 = cp.tile([S, H], F32R)
            nc.sync.dma_start(dw1[:], d["dw1"][:])
            dw2 = cp.tile([128, 4 * F], F32R)
            nc.sync.dma_start(
                dw2[:].rearrange("k (g m) -> k g m", g=4),
                d["dw2"][:].rearrange("(g k) m -> k g m", g=4))
            db1 = cp.tile([128, 4], F32)
            nc.sync.dma_start(db1[:], d["db1"][:])
            db2 = cp.tile([128, 4], F32)
            nc.sync.dma_start(db2[:], d["db2"][:])
            m2wm = cp.tile([128, 2 * S * S], F32R)
            nc.sync.dma_start(m2wm[:], d["m2w"][:])
            b2p = cp.tile([128, NBLK], F32)
            nc.sync.dma_start(b2p[:], d["b2p"][:])
            sT = cp.tile([128, NBLK], F32)
            klp = cp.tile([S, 3], F32)
            rsq = cp.tile([128, 4], F32)
            zTe = cp.tile([S + 1, BL], F32R)

            # ---- encoder layer 1: hT = lrelu(W1^T xT + b1) ----
            with tc.tile_pool(name="encw", bufs=1) as ewp:
                m1wm = ewp.tile([S + 1, S * H2], F32R)
                nc.sync.dma_start(m1wm[:], d["m1w"][:])
                hT = [cp.tile([128, BL], F32R, name=f"hT{m}", tag=f"hT{m}") for m in range(4)]
                for mt in range(4):
                    p_ = psA.tile([128, BL], F32, tag="pt", bufs=2)
                    for kt in range(4):
                        nc.tensor.matmul(
                            p_[:], ew1[:, 4 * 128 * kt + mt * 128:
                                        4 * 128 * kt + (mt + 1) * 128]
                            .rearrange("k m -> k m"),
                            xt[kt][:], start=(kt == 0), stop=(kt == 3))
                    nc.scalar.activation(hT[mt][:], p_[:], AF.Prelu,
                                         bias=eb1[:, mt:mt + 1], alpha=LEAK)

                # ---- encoder layer 2: params = W2^T hT + b2 ----
                pp = psA.tile([2 * S, BL], F32, tag="pt", bufs=2)
                for kt in range(4):
                    nc.tensor.matmul(
                        pp[:], ew2[:, kt * 2 * S:(kt + 1) * 2 * S],
                        hT[kt][:], start=(kt == 0), stop=(kt == 3))
                meanT = cp.tile([S, BL], F32R)
                stdT = cp.tile([S, BL], F32R)
                nc.vector.tensor_scalar_add(meanT[:], pp[0:S, :], eb2[:, 0:1])
                nc.scalar.activation(stdT[:], pp[S:2 * S, :], AF.Exp,
                                     bias=eb2[:, 1:2])
                nc.sync.dma_start(d["meanT"][:], meanT[:])
                nc.sync.dma_start(d["stdT"][:], stdT[:])
                # kl partial sums (scratch outputs unused); eb2 col2 = 2*ls bias
                ksc = scp.tile([S, BL], F32, tag="ksc")
                nc.scalar.activation(ksc[:], pp[0:S, :], AF.Square,
                                     bias=eb2[:, 0:1], accum_out=klp[:, 0:1])
                ksc3 = scp.tile([S, BL], F32, tag="ksc")
                nc.scalar.activation(ksc3[:], pp[S:2 * S, :], AF.Exp, scale=2.0,
                                     bias=eb2[:, 2:3], accum_out=klp[:, 1:2])
                ksc4 = scp.tile([S, BL], F32, tag="ksc")
                nc.scalar.activation(ksc4[:], pp[S:2 * S, :], AF.Identity,
                                     bias=eb2[:, 1:2], accum_out=klp[:, 2:3])
                nc.sync.dma_start(d["klp"][:], klp[:])

                # ---- z = mean + std * eps ; append ones row ----
                nc.vector.tensor_tensor(zTe[0:S, :], stdT[:], epsT[:], ALU.mult)
                nc.vector.tensor_tensor(zTe[0:S, :], zTe[0:S, :], meanT[:], ALU.add)
                nc.vector.tensor_scalar(zTe[S:S + 1, :], epsT[0:1, :], 0.0, 1.0,
                                        ALU.mult, ALU.add)

                # ---- mech + MI: one interleaved two-stage pipeline ----
                # Unit = either one mech i (two 128-ch chunks) or one MI
                # double-block (two 2-pair blocks). Stage1 = matmuls into a
                # [128, 2*BL] psum + one leaky eviction (ACT Prelu or DVE
                # mul+max, round-robin for engine balance). Stage2 (emitted
                # DEPTH units behind its stream) = the consuming matmuls +
                # ACT evict. Interleaving keeps ACT/DVE fed from the start.
                DEPTH = 2
                NDBL = NBLK // 2
                zcp = psB.tile([S, BL], F32, tag="zcp")
                zTeb = cp.tile([S + 1, BL], F32R)
                h1ds, m1ds, g1s, g2s = {}, {}, {}, {}
                flexctr = [0]

                def leaky_evict(dst, srcp):
                    k = flexctr[0]
                    flexctr[0] += 1
                    if k % 2 == 0:
                        nc.scalar.activation(dst[:], srcp[:], AF.Prelu, alpha=LEAK)
                    else:
                        t0 = scp.tile([128, 2 * BL], F32R, tag="t0")
                        nc.vector.tensor_scalar_mul(t0[:], srcp[:], LEAK)
                        nc.vector.tensor_tensor(dst[:], srcp[:], t0[:], ALU.max)

                def mech_stage1(u):
                    hp = psA.tile([128, 2 * BL], F32, tag="pt", bufs=2)
                    for c in range(2):
                        nc.tensor.matmul(
                            hp[:, c * BL:(c + 1) * BL],
                            m1wm[:, u * H2 + c * 128: u * H2 + (c + 1) * 128],
                            zTe[:], start=True, stop=True)
                    h1d = m1p.tile([128, 2 * BL], F32R, tag="h1t", bufs=DEPTH + 2)
                    leaky_evict(h1d, hp)
                    h1ds[u] = h1d

                def mech_stage2(u):
                    h1d = h1ds.pop(u)
                    for c in range(2):
                        j = 2 * u + c
                        nc.tensor.matmul(
                            zcp[:], m2wm[:, j * S:(j + 1) * S],
                            h1d[:, c * BL:(c + 1) * BL],
                            start=(j == 0), stop=(j == 2 * S - 1))

                def mi_stage1(u):
                    for h in range(2):
                        b = 2 * u + h
                        g, j = divmod(b, GRP)
                        if j == 0:
                            g1 = g1p.tile([S + 1, GRP * 128], F32R, tag="g1")
                            nc.sync.dma_start(g1[:], d["mi1w"][g])
                            g2 = g2p.tile([128, GRP * 128], F32R, tag="g2")
                            nc.sync.dma_start(g2[:], d["mi2w"][g])
                            g1s[g] = g1
                            g2s[g] = g2
                    mp1 = psA.tile([128, 2 * BL], F32, tag="pt", bufs=2)
                    for h in range(2):
                        b = 2 * u + h
                        g, j = divmod(b, GRP)
                        nc.tensor.matmul(mp1[:, h * BL:(h + 1) * BL],
                                         g1s[g][:, j * 128:(j + 1) * 128],
                                         zTeb[:], start=True, stop=True)
                    m1d = m1p.tile([128, 2 * BL], F32R, tag="m1t", bufs=DEPTH + 2)
                    leaky_evict(m1d, mp1)
                    m1ds[u] = m1d

                def mi_stage2(u):
                    m1d = m1ds.pop(u)
                    for h in range(2):
                        b = 2 * u + h
                        g, j = divmod(b, GRP)
                        mp2 = psA.tile([128, BL], F32, tag="pt2", bufs=3)
                        nc.tensor.matmul(mp2[:], g2s[g][:, j * 128:(j + 1) * 128],
                                         m1d[:, h * BL:(h + 1) * BL],
                                         start=True, stop=True)
                        nc.scalar.activation(mp2[:], mp2[:], AF.Prelu,
                                             bias=b2p[:, b:b + 1], alpha=LEAK,
                                             accum_out=sT[:, b:b + 1])

                nc.vector.tensor_copy(zTeb[:], zTe[:])
                # interleave schedule: one mech unit every 5th slot
                sched = []
                mi_i = me_i = 0
                while mi_i < NDBL or me_i < S:
                    if (len(sched) % 5 == 4 and me_i < S) or mi_i >= NDBL:
                        sched.append(("mech", me_i)); me_i += 1
                    else:
                        sched.append(("mi", mi_i)); mi_i += 1
                done1 = []
                for kind, u in sched:
                    (mech_stage1 if kind == "mech" else mi_stage1)(u)
                    done1.append((kind, u))
                    if len(done1) > DEPTH:
                        k2, u2 = done1[len(done1) - 1 - DEPTH]
                        (mech_stage2 if k2 == "mech" else mi_stage2)(u2)
                for k2, u2 in done1[len(done1) - DEPTH:]:
                    (mech_stage2 if k2 == "mech" else mi_stage2)(u2)
                nc.sync.dma_start(d["sT"][:], sT[:])

                zcT = cp.tile([S, BL], F32R)
                nc.vector.tensor_scalar_add(zcT[:], zcp[:], mb2[:])
                nc.sync.dma_start(d["zcT"][:], zcT[:])

            # ---- decoder ----
            d1T = [cp.tile([128, BL], F32R, name=f"d1T{m}", tag=f"d1T{m}") for m in range(4)]
            for mt in range(4):
                p_ = psA.tile([128, BL], F32, tag="pt", bufs=2)
                nc.tensor.matmul(p_[:], dw1[:, mt * 128:(mt + 1) * 128],
                                 zcT[:], start=True, stop=True)
                nc.scalar.activation(d1T[mt][:], p_[:], AF.Prelu,
                                     bias=db1[:, mt:mt + 1], alpha=LEAK)
            for mt in range(4):
                p_ = psA.tile([128, BL], F32, tag="pt", bufs=2)
                for kt in range(4):
                    nc.tensor.matmul(
                        p_[:], dw2[:, 4 * 128 * kt + mt * 128:
                                    4 * 128 * kt + (mt + 1) * 128]
                        .rearrange("k m -> k m"),
                        d1T[kt][:], start=(kt == 0), stop=(kt == 3))
                rt = scp.tile([128, BL], F32R, tag="rt")
                nc.vector.tensor_scalar_add(rt[:], p_[:], db2[:, mt:mt + 1])
                nc.sync.dma_start(d["reconT"][mt * 128:(mt + 1) * 128, :], rt[:])
                df = scp.tile([128, BL], F32R, tag="df")
                nc.vector.tensor_tensor(df[:], rt[:], xt[mt][:], ALU.subtract)
                dsc = scp.tile([128, BL], F32, tag="dsc")
                nc.scalar.activation(dsc[:], df[:], AF.Square,
                                     accum_out=rsq[:, mt:mt + 1])
            nc.sync.dma_start(d["rsq"][:], rsq[:])

    nc.compile()
    return nc


def _pack_host(inputs):
    """Build per-core input maps from full inputs."""
    f32 = np.float32
    g = {k: np.asarray(v, dtype=f32) for k, v in inputs.items()}
    lw = g["log_weight"]
    eye = np.eye(S, dtype=f32)
    upper = np.triu(np.ones((S, S), f32), k=1)
    x_ = (lw * upper - 10.0 * eye) / TEMP
    A = np.where(
        x_ >= 0, 1.0 / (1.0 + np.exp(-np.clip(x_, 0, None))),
        np.exp(np.clip(x_, None, 0)) / (1.0 + np.exp(np.clip(x_, None, 0)))
    ).astype(f32)

    # mech layer-1 lhsT with A absorbed + bias row: [S+1, S*H2]
    w1a = np.einsum("si,ish->ish", A, g["mech_w1"])  # w~1[i,s,h] = A[s,i]*w1[i,s,h]
    m1w = np.zeros((S + 1, S * H2), f32)
    m1w[0:S] = w1a.transpose(1, 0, 2).reshape(S, S * H2)
    m1w[S] = g["mech_b1"].reshape(S * H2)
    # mech layer-2 lhsT: [128, 2*S*S]; col block j=(i,c): column i = w2[i, c*128:...]
    m2w = np.zeros((128, 2 * S, S), f32)
    ii = np.arange(S)
    for c in range(2):
        m2w[:, 2 * ii + c, ii] = g["mech_w2"][:, c * 128:(c + 1) * 128].T
    m2w = m2w.reshape(128, 2 * S * S)

    iu, ju = np.triu_indices(S, k=1)
    # MI layer-1 sparse lhsT groups [NGRP, S+1, GRP*128]
    mi1 = np.zeros((NBLK, S + 1, 128), f32)
    pw = g["mi_w1"]  # [P, 2, CH]
    pb = g["mi_b1"]  # [P, CH]
    bi = np.arange(NBLK)[:, None]
    ci = np.arange(CH)[None, :]
    for h in range(2):
        p_idx = 2 * np.arange(NBLK) + h
        cols = slice(h * CH, (h + 1) * CH)
        mi1[bi, iu[p_idx][:, None], ci + h * CH] = pw[p_idx, 0]
        mi1[bi, ju[p_idx][:, None], ci + h * CH] += pw[p_idx, 1]
        mi1[:, S, cols] = pb[p_idx]
    mi1w = mi1.reshape(NGRP, GRP, S + 1, 128).transpose(0, 2, 1, 3) \
              .reshape(NGRP, S + 1, GRP * 128).copy()
    # MI layer-2 block-diag groups [NGRP, 128, GRP*128]
    mi2 = np.zeros((NBLK, 128, 128), f32)
    mi2[:, 0:CH, 0:CH] = g["mi_w2"][0::2]
    mi2[:, CH:128, CH:128] = g["mi_w2"][1::2]
    mi2w = mi2.reshape(NGRP, GRP, 128, 128).transpose(0, 2, 1, 3) \
              .reshape(NGRP, 128, GRP * 128).copy()
    b2p = np.concatenate([g["mi_b2"][0::2], g["mi_b2"][1::2]], axis=1).T.copy()
    # ^ [128, NBLK]: rows 0:64 pair 2b bias, 64:128 pair 2b+1

    eb2 = np.stack([g["enc_b2"][0:S], g["enc_b2"][S:2 * S],
                    2.0 * g["enc_b2"][S:2 * S]], axis=1)

    def pack_bias_cols(bvec):  # [512] -> [128, 4]
        return bvec.reshape(4, 128).T.copy()

    shared = {
        "ew1": g["enc_w1"], "ew2": g["enc_w2"],
        "eb1": pack_bias_cols(g["enc_b1"]), "eb2": eb2,
        "m1w": m1w, "m2w": m2w, "mb2": g["mech_b2"][:, None],
        "dw1": g["dec_w1"], "dw2": g["dec_w2"],
        "db1": pack_bias_cols(g["dec_b1"]), "db2": pack_bias_cols(g["dec_b2"]),
        "mi1w": mi1w, "mi2w": mi2w, "b2p": b2p,
    }
    shared = {k: np.ascontiguousarray(v, dtype=f32) for k, v in shared.items()}

    in_maps = []
    for c in range(NCORES):
        sl = slice(c * BL, (c + 1) * BL)
        m = dict(shared)
        m["xT"] = np.ascontiguousarray(g["features"][sl].T)
        m["epsT"] = np.ascontiguousarray(g["eps"][sl].T)
        in_maps.append(m)
    return in_maps, g, A, (iu, ju)


def kernel(**inputs):
    if "nc" not in _CACHE:
        _CACHE["nc"] = _build_nc()
    nc = _CACHE["nc"]

    in_maps, g, A, (iu, ju) = _pack_host(inputs)
    res = run_bass_kernel_spmd(nc, in_maps, list(range(NCORES))).results

    f32 = np.float32
    mean = np.concatenate([r["meanT"].T for r in res], axis=0)
    std = np.concatenate([r["stdT"].T for r in res], axis=0)
    zc = np.concatenate([r["zcT"].T for r in res], axis=0)
    recon = np.concatenate([r["reconT"].T for r in res], axis=0)

    klp = sum(r["klp"] for r in res)          # [S, 3]
    rsq_tot = float(sum(r["rsq"].sum() for r in res))
    sT = sum(r["sT"].astype(np.float64) for r in res)  # [128, NBLK]

    recon_loss = rsq_tot / (B * F)
    kl_loss = 0.5 * (klp[:, 0].sum() + klp[:, 1].sum()
                     - 2.0 * klp[:, 2].sum() - B * S) / B

    lw = g["log_weight"]
    sig = 1.0 / (1.0 + np.exp(-lw))
    sparsity_loss = float(np.abs(sig).sum())

    # mi_est[p] = (w3[p] . sum_b m2[p]) / B + b3[p]
    s_pairs = np.empty((P, CH), np.float64)
    s_pairs[0::2] = sT[0:CH].T
    s_pairs[1::2] = sT[CH:128].T
    mi_est = (s_pairs * g["mi_w3"]).sum(axis=1) / B + g["mi_b3"]
    mi_loss = float((sig[iu, ju] * mi_est).sum())

    A2 = (sig * sig).astype(f32)
    expm = np.eye(S, dtype=f32)
    mp = np.eye(S, dtype=f32)
    fact = 1.0
    for i in range(1, 10):
        fact *= i
        mp = mp @ A2
        expm = expm + mp / fact
    dag_loss = float(np.trace(expm) - S) ** 2

    total = recon_loss + kl_loss + 0.1 * sparsity_loss + 0.01 * mi_loss + 1.0 * dag_loss

    def sc(x):
        return np.float32(x)

    return (zc.astype(f32), mean.astype(f32), std.astype(f32), recon.astype(f32),
            sc(total), sc(recon_loss), sc(kl_loss), sc(sparsity_loss),
            sc(mi_loss), sc(dag_loss))
